# revision 22
# baseline (speedup 1.0000x reference)
"""Causal self-attention (B=4, T=2048, E=768, H=12, D=64) on 8 TRN2 NeuronCores.

Sharding: core c handles batch b = c//2 and head-group g = c%2 (6 heads each).
Per core:
    qT, kT = (x @ WqT + bq).T, ...        stored [384, 2048] (heads x 64, T)
    v      = x @ WvT + bv, scaled by w=exp(mask), stored fp8 in strip-pair
             interleaved tiles [128, 2 strips, 6 heads x 65] with a w column
    per head-pair, per key-strip sb (128 keys), per 512-col piece:
        scoresT[s, t] for BOTH heads into ONE psum tile [128, 1024]
        (h0 cols 0-511, h1 cols 512-1023) via four 64x64-tiled matmuls that
        run concurrently in the PE array.
        causal mask: DVE adds -1920 to the masked lanes of the diagonal
        128x128 block directly in psum (exp then underflows to 0).
        ONE exp op per piece covers both heads through a strided 3D out AP,
        writing fp8e4 into the strip-pair interleaved expT buffer.
    per head, per t-window (512 cols): PV via fp8 DoubleRow matmuls, one per
        strip-PAIR (256-wide contraction: v tiles hold strip pairs at j-stride
        400, expT holds them at j-stride Wp). Psum [65, W] accumulates over
        pairs; row 64 (w column) is the softmax denominator. Window result is
        DMA'd to DRAM directly from psum.
Host: output[b, :, h*64:(h+1)*64] = (outT_h[:64] / outT_h[64:65]).T

attention_mask is folded in as w_s = exp(mask_s): v' = w*v and the ones
column becomes w, so softmax numerator and denominator both carry w exactly.
All matmul start=True writes begin on a PSUM bank boundary. Inputs are
host-packed into the exact SBUF layouts so every input lands in a handful of
contiguous 2D DMAs on two queues.
"""

import numpy as np
import ml_dtypes

import concourse.bacc as bacc
import concourse.mybir as mybir
import concourse.tile as tile
from concourse import bass_utils

F32 = mybir.dt.float32
BF16 = mybir.dt.bfloat16
FP8 = mybir.dt.float8e4

B, T, E, H, D = 4, 2048, 768, 12, 64
NCORES = 8
HPC = 6             # heads per core
OC = HPC * D        # 384 output channels per core
ECH = E // 128      # 6 contraction chunks
QKC = OC // 128     # 3 qT/kT partition chunks (= head pairs)
NSB = T // 128      # 16 key strips
NPB = NSB // 2      # 8 strip pairs
SCALE = 0.125       # 1/sqrt(D)
XSL = ECH * 512     # xT free elems per t-slice (3072)
VJS = 400           # vtp j-stride (>= 6*65, 16B-aligned for DoubleRow)
MNEG = -1920.0      # causal mask bias: SCALE*(-1920) = -240 -> exp == 0

# strip-pair expT regions: pair pb holds strips (2pb, 2pb+1) for both heads
# of the current attention pair. Region for (pb, hh) is [2, Wp] fp8 at
# OFF[pb] + 2*Wp*hh; j-row 1 (odd strip) is absolute-t aligned, so its first
# 128 cols are a never-written zero pad.
PW = [T - 256 * pb for pb in range(NPB)]
OFF = [0] * (NPB + 1)
for _pb in range(NPB):
    OFF[_pb + 1] = OFF[_pb] + 4 * PW[_pb]
EXBW = OFF[NPB]  # 36864


def _strip_pieces(sb):
    """(piece_idx, start, width) pieces of strip sb on the 512 grid."""
    W = T - 128 * sb
    return [(p // 512, p, min(512, W - p)) for p in range(0, W, 512)]


def _xoff(e, t):
    """Free-dim offset of (e-chunk, t) in the slice-major packed xT tile."""
    return (t // 512) * XSL + 512 * e + (t % 512)


def _build():
    nc = bacc.Bacc("TRN2", debug=False)

    xT_d = nc.dram_tensor("xTp", [128, ECH * T], BF16, kind="ExternalInput")
    wq_d = nc.dram_tensor("wqp", [128, ECH * OC], BF16, kind="ExternalInput")
    wk_d = nc.dram_tensor("wkp", [128, ECH * OC], BF16, kind="ExternalInput")
    wv_d = nc.dram_tensor("wvp", [128, ECH * OC], BF16, kind="ExternalInput")
    bq_d = nc.dram_tensor("bq", [QKC, 128, 1], F32, kind="ExternalInput")
    bk_d = nc.dram_tensor("bk", [QKC, 128, 1], F32, kind="ExternalInput")
    bvr_d = nc.dram_tensor("bvr", [128, OC], F32, kind="ExternalInput")
    wpk_d = nc.dram_tensor("wpk", [128, 8 * NSB], F32, kind="ExternalInput")
    trin_d = nc.dram_tensor("trin", [128, 256], F32, kind="ExternalInput")
    out_d = nc.dram_tensor("outT", [HPC, D + 1, T], F32, kind="ExternalOutput")

    with tile.TileContext(nc) as tc:
        with (
            tc.tile_pool(name="persist", bufs=1) as pp,
            tc.tile_pool(name="qk_ps", bufs=3, space="PSUM") as qk_ps,
            tc.tile_pool(name="b1_ps", bufs=2, space="PSUM") as b1_ps,
            tc.tile_pool(name="stage", bufs=4) as stage,
        ):
            # ---- persistent SBUF tensors ----
            xt_all = pp.tile([128, ECH * T], BF16, tag="xt", name="xt")
            wq_all = pp.tile([128, ECH * OC], BF16, tag="wq", name="wq")
            wk_all = pp.tile([128, ECH * OC], BF16, tag="wk", name="wk")
            wv_all = pp.tile([128, ECH * OC], BF16, tag="wv", name="wv")
            wv = [wv_all[:, OC * e:OC * (e + 1)] for e in range(ECH)]

            def wqk_sl(w_all, c, e):
                o = 768 * c + 128 * e
                return w_all[:, o:o + 128]
            qt = [pp.tile([128, T], BF16, tag=f"qt{c}", name=f"qt{c}") for c in range(QKC)]
            kt = [pp.tile([128, T], BF16, tag=f"kt{c}", name=f"kt{c}") for c in range(QKC)]
            vtp = [pp.tile([128, 2, VJS], FP8, tag=f"vt{p}", name=f"vt{p}") for p in range(NPB)]
            # fp8 residuals of v for pairs 0-1: corrects the first t-window,
            # where attention is peaked and out ~= v so v's fp8 rounding
            # would land directly in the output
            vlo = [pp.tile([128, 2, VJS], FP8, tag=f"vl{p}", name=f"vl{p}") for p in range(2)]
            # bf16 side path for output cols t < 256: with few keys the fp8
            # rounding of exp perturbs softmax weights directly (no averaging)
            vt16 = [pp.tile([128, HPC, 65], BF16, tag=f"v6{p}", name=f"v6{p}") for p in range(2)]
            exb16 = pp.tile([128, 2, 384], BF16, tag="ex6", name="ex6")
            exb = pp.tile([128, EXBW], FP8, tag="exb", name="exb")
            bq_t = [pp.tile([128, 1], F32, tag=f"bq{c}", name=f"bq{c}") for c in range(QKC)]
            bk_t = [pp.tile([128, 1], F32, tag=f"bk{c}", name=f"bk{c}") for c in range(QKC)]
            bvr_t = pp.tile([128, OC], F32, tag="bvr", name="bvr")
            wpk_t = pp.tile([128, 8 * NSB], F32, tag="wpk", name="wpk")
            trin_t = pp.tile([128, 256], F32, tag="trin", name="trin")

            # ---- input DMAs: contiguous 2D transfers on two hardware queues
            # (sync + scalar); xT slice-major so projections start early ----
            def dma_w(eng, w_all, w_d, c):
                eng.dma_start(w_all[:, 768 * c:768 * c + 768],
                              w_d.ap()[:, 768 * c:768 * c + 768])

            def dma_x(eng, t0):
                s0 = (t0 // 512) * XSL
                eng.dma_start(xt_all[:, s0:s0 + XSL], xT_d.ap()[:, s0:s0 + XSL])

            def dma_xh(eng, t0, half):
                # half a 512-col t-slice of xT (splitting the first slices
                # across two queues halves the time to the first projection)
                s0 = (t0 // 512) * XSL + half * (XSL // 2)
                eng.dma_start(xt_all[:, s0:s0 + XSL // 2],
                              xT_d.ap()[:, s0:s0 + XSL // 2])

            dma_w(nc.sync, wk_all, wk_d, 0)
            dma_xh(nc.sync, 0, 0)
            dma_w(nc.scalar, wq_all, wq_d, 0)
            dma_xh(nc.scalar, 0, 1)
            dma_xh(nc.sync, 512, 0)
            dma_xh(nc.scalar, 512, 1)
            dma_w(nc.sync, wk_all, wk_d, 1)
            dma_w(nc.scalar, wq_all, wq_d, 1)
            dma_xh(nc.sync, 1024, 0)
            dma_xh(nc.scalar, 1024, 1)
            dma_xh(nc.sync, 1536, 0)
            dma_xh(nc.scalar, 1536, 1)
            dma_w(nc.sync, wk_all, wk_d, 2)
            dma_w(nc.scalar, wq_all, wq_d, 2)
            nc.gpsimd.dma_start(trin_t[:, :], trin_d.ap()[:, :])
            for c in range(QKC):
                nc.gpsimd.dma_start(bq_t[c][:, :], bq_d.ap()[c])
                nc.gpsimd.dma_start(bk_t[c][:, :], bk_d.ap()[c])
            nc.gpsimd.dma_start(wpk_t[:, :], wpk_d.ap()[:, :])
            nc.gpsimd.dma_start(wv_all[:, :], wv_d.ap()[:, :])
            nc.gpsimd.dma_start(bvr_t[:, :], bvr_d.ap()[:, :])

            # one-time zero pad: j=1 row's first 128 cols of every region
            # (on gpsimd: keeps the DVE queue free for the projection path)
            for pb in range(NPB):
                for hh in (0, 1):
                    o = OFF[pb] + 2 * PW[pb] * hh + PW[pb]
                    nc.gpsimd.memset(exb[:, o:o + 128], 0.0)
            # zero v tiles: stale fp8 bytes can encode NaN, and NaN*0 = NaN
            # even where the exp operand is a structural zero
            for t_ in vtp + vlo:
                nc.gpsimd.memset(t_[:, :, :], 0.0)

            def proj_qk_chain(c, t0, which):
                # one 512-col chain of the qT or kT projection for chunk c
                w_all, dst, bias = ((wk_all, kt, bk_t), (wq_all, qt, bq_t))[which]
                ps = b1_ps.tile([128, 512], F32, tag="b1", name="pp")
                for e in range(ECH):
                    nc.tensor.matmul(
                        ps[:, :],
                        wqk_sl(w_all, c, e),
                        xt_all[:, _xoff(e, t0):_xoff(e, t0) + 512],
                        start=(e == 0), stop=(e == ECH - 1),
                    )
                nc.vector.tensor_scalar_add(
                    dst[c][:, t0:t0 + 512], ps[:, :], bias[c][:, 0:1])

            def proj_v_chain(tb):
                pb, dlt = divmod(tb, 2)
                ps = b1_ps.tile([128, OC], F32, tag="b1", name="ppv")
                for e in range(ECH):
                    o = _xoff(e, 128 * tb)
                    nc.tensor.matmul(
                        ps[:, :],
                        xt_all[:, o:o + 128],
                        wv[e][:, :],
                        start=(e == 0), stop=(e == ECH - 1),
                    )
                # v' = (psum + bv) * w, fp8; w column = w  (w = exp(mask))
                nc.vector.tensor_tensor(
                    ps[:, :], ps[:, :], bvr_t[:, :], op=mybir.AluOpType.add)
                nc.vector.tensor_scalar_mul(
                    ps[:, :], ps[:, :], wpk_t[:, 8 * tb:8 * tb + 1])
                dst = vtp[pb][:, dlt, 0:HPC * 65].rearrange(
                    "p (h d) -> p h d", h=HPC)
                nc.vector.tensor_copy(
                    dst[:, :, 0:D], ps.rearrange("p (h d) -> p h d", h=HPC))
                nc.vector.tensor_copy(
                    dst[:, :, D:D + 1],
                    wpk_t[:, 8 * tb + 1:8 * tb + 7].rearrange(
                        "p (h d) -> p h d", d=1))
                if tb < 4:
                    # residual: vlo = fp8(v' - float(fp8(v')))
                    lo = vlo[pb][:, dlt, 0:HPC * 65].rearrange(
                        "p (h d) -> p h d", h=HPC)
                    nc.vector.tensor_tensor(
                        lo[:, :, 0:D],
                        ps.rearrange("p (h d) -> p h d", h=HPC),
                        dst[:, :, 0:D], op=mybir.AluOpType.subtract)
                if tb < 2:
                    nc.vector.tensor_copy(
                        vt16[tb][:, :, 0:D],
                        ps.rearrange("p (h d) -> p h d", h=HPC))
                    nc.vector.tensor_copy(
                        vt16[tb][:, :, D:D + 1],
                        wpk_t[:, 8 * tb + 1:8 * tb + 7].rearrange(
                            "p (h d) -> p h d", d=1))

            def qk_exp_piece(h0, sb, p, rp, pw):
                # piece p of strip sb for the pair (h0, h0+1): two adjacent
                # head blocks in one psum tile (h0 cols 0-511, h1 512-1023)
                # via 4 concurrent 64x64-tiled matmuls, then causal-mask the
                # diagonal block in psum (p==0) and ONE strided exp -> fp8.
                c = h0 // 2
                t0 = 128 * sb
                ps = qk_ps.tile([128, 1024], F32, tag="qk", name="qk")
                for ofs, rows in ((0, slice(0, 64)), (512, slice(64, 128))):
                    for so, pr in ((0, slice(0, 64)), (64, slice(64, 128))):
                        nc.tensor.matmul(
                            ps[pr, ofs:ofs + pw],
                            kt[c][rows, t0 + so:t0 + so + 64],
                            qt[c][rows, t0 + rp:t0 + rp + pw],
                            start=True, stop=True,
                        )
                ps3 = ps.rearrange("p (h c) -> p h c", h=2)
                if p == 0:
                    # diag 128x128 blocks (cols 0-127 and 512-639): add -1920
                    # where key > query so exp underflows to exact 0
                    nc.vector.tensor_tensor(
                        ps3[:, :, 0:128], ps3[:, :, 0:128],
                        trin_t.rearrange("p (h c) -> p h c", h=2),
                        op=mybir.AluOpType.add)
                pb, dlt = divmod(sb, 2)
                Wp = PW[pb]
                d0 = dlt * (Wp + 128) + rp
                dst = exb[:, OFF[pb]:OFF[pb] + 4 * Wp].rearrange(
                    "p (h c) -> p h c", h=2)[:, :, d0:d0 + pw]
                nc.scalar.activation(
                    dst, ps3[:, :, 0:pw],
                    mybir.ActivationFunctionType.Exp, scale=SCALE)
                if sb <= 1 and p == 0:
                    # bf16 exp copy of the t < 256 cols for the side path
                    w16 = 256 if sb == 0 else 128
                    o16 = 0 if sb == 0 else 256
                    nc.scalar.activation(
                        exb16[:, :, o16:o16 + w16], ps3[:, :, 0:w16],
                        mybir.ActivationFunctionType.Exp, scale=SCALE)

            _oq = [0]

            def out_queue():
                _oq[0] += 1
                return (nc.sync, nc.scalar, nc.gpsimd)[_oq[0] % 3]

            def pv_window(h, c0, W):
                # one [65, W] psum accumulation over strip-pairs via fp8
                # DoubleRow (256-wide contraction), then DMA straight out.
                hh = h % 2
                ps = b1_ps.tile([D + 1, W], F32, tag="b1", name="pv",
                                padded_shape=[D + 1, 512])
                segs = [(pb, vtp[pb]) for pb in range(NPB) if 256 * pb < c0 + W]
                if c0 < 512:
                    # residual correction for the first windows (peaked rows)
                    segs += [(pb, vlo[pb]) for pb in range(2) if 256 * pb < c0 + W]
                for i, (pb, vt_) in enumerate(segs):
                    s0 = max(c0, 256 * pb)
                    w = c0 + W - s0
                    Wp = PW[pb]
                    o = OFF[pb] + 2 * Wp * hh
                    rhs = exb[:, o:o + 2 * Wp].rearrange(
                        "p (j c) -> p j c", j=2)[:, :, s0 - 256 * pb:s0 - 256 * pb + w]
                    nc.tensor.matmul(
                        ps[:, s0 - c0:s0 - c0 + w],
                        vt_[:, :, 65 * h:65 * h + 65],
                        rhs,
                        start=(i == 0), stop=(i == len(segs) - 1),
                        perf_mode=mybir.MatmulPerfMode.DoubleRow,
                    )
                st = stage.tile([D + 1, W], F32, tag="st", name="st",
                                padded_shape=[D + 1, 512])
                nc.vector.tensor_copy(st[:, :], ps[:, :])
                out_queue().dma_start(out_d.ap()[h, :, c0:c0 + W], st[:, :])

            def pv16_window(h):
                # bf16 side path for output cols [0, 256): strips 0 and 1
                hh = h % 2
                ps = b1_ps.tile([D + 1, 256], F32, tag="b1", name="pv6",
                                padded_shape=[D + 1, 512])
                nc.tensor.matmul(
                    ps[:, 0:256], vt16[0][:, h, :], exb16[:, hh, 0:256],
                    start=True, stop=False)
                nc.tensor.matmul(
                    ps[:, 128:256], vt16[1][:, h, :], exb16[:, hh, 256:384],
                    start=False, stop=True)
                st = stage.tile([D + 1, 256], F32, tag="st", name="st6",
                                padded_shape=[D + 1, 512])
                nc.vector.tensor_copy(st[:, :], ps[:, :])
                out_queue().dma_start(out_d.ap()[h, :, 0:256], st[:, :])

            # pv windows: strip -> [(c0, W)]. Tail windows split so most of
            # the last chunk's accumulation runs before the final strips.
            PV_WINDOWS = {3: [(256, 256)], 7: [(512, 512)], 11: [(1024, 512)],
                          13: [(1536, 256)], 14: [(1792, 128)],
                          15: [(1920, 128)]}

            # ---- pipelined emission: a global slot pipeline ----
            # Pair p's strips occupy slots [12p, 12p+16); consecutive pairs
            # overlap by 4 slots so ACT never idles at pair boundaries. A
            # piece whose expT region is still to be read by the previous
            # pair's late pv windows (t >= 1536) is deferred past them.
            NSLOT = 41
            slot_work = [[] for _ in range(NSLOT)]   # items: (kind, fn)
            # kind 0: scores pieces; 1: proj fillers; 2: pv windows

            # pair (0,1) cascaded start: emit chunk-0 projection chains
            # t-ascending; after each chain emit every scores piece whose q/k
            # columns are available, so the first exp fires early.
            all_pieces = [(128 * sb + rp + pw, sb, p, rp, pw)
                          for sb in range(NSB)
                          for p, rp, pw in _strip_pieces(sb)]
            all_pieces.sort(key=lambda x: (x[0], x[1]))
            emitted = set()

            def emit_ready(limit):
                for need, sb, p, rp, pw in all_pieces:
                    if need > limit:
                        break
                    if (sb, p) in emitted:
                        continue
                    qk_exp_piece(0, sb, p, rp, pw)
                    emitted.add((sb, p))

            for t0 in range(0, T, 512):
                proj_qk_chain(0, t0, 0)
                proj_qk_chain(0, t0, 1)
                emit_ready(min(t0 + 512, 1024))

            def sched_pair(h0, base, defer_slot):
                for sb in range(NSB):
                    for p, rp, pw in _strip_pieces(sb):
                        if h0 == 0 and (sb, p) in emitted:
                            continue
                        t_end = 128 * sb + rp + pw
                        sl = base + sb
                        if t_end > 1536 and sl < defer_slot:
                            sl = defer_slot
                        slot_work[sl].append(
                            (0, lambda h0=h0, sb=sb, p=p, rp=rp, pw=pw:
                             qk_exp_piece(h0, sb, p, rp, pw)))
                    if sb == 2:
                        slot_work[base + 2].append(
                            (2, lambda h0=h0: pv16_window(h0)))
                        slot_work[base + 2].append(
                            (2, lambda h0=h0: pv16_window(h0 + 1)))
                    for tc0, W in PV_WINDOWS.get(sb, ()):
                        sl = max(base + sb, defer_slot if base else 0)
                        slot_work[sl].append(
                            (2, lambda h0=h0, tc0=tc0, W=W: pv_window(h0, tc0, W)))
                        slot_work[sl].append(
                            (2, lambda h0=h0, tc0=tc0, W=W: pv_window(h0 + 1, tc0, W)))

            sched_pair(0, 0, 0)
            sched_pair(2, 12, 16)
            sched_pair(4, 24, 28)
            # fillers: v projection paced over pair01's strips (1/slot, just
            # ahead of the pv windows); chunk-1/2 q/k projections early
            # block 15 goes to slot 14: pair 0's [1792,1920) window (slot 14)
            # touches vtp[7] (its exp side is the structural zero pad, but the
            # v bytes must be initialized before any matmul reads them)
            for tb in range(NSB):
                slot_work[min(tb, 14)].append((1, lambda tb=tb: proj_v_chain(tb)))
            for i in range(8):
                t0, wch = (i % 4) * 512, i // 4
                slot_work[1 + i].append(
                    (1, lambda t0=t0, w=wch: proj_qk_chain(1, t0, w)))
                slot_work[13 + i].append(
                    (1, lambda t0=t0, w=wch: proj_qk_chain(2, t0, w)))

            for sl in range(NSLOT):
                for _, f in sorted(slot_work[sl], key=lambda kf: kf[0]):
                    f()

    nc.compile()
    return nc


_NC_CACHE = None


def _get_nc():
    global _NC_CACHE
    if _NC_CACHE is None:
        _NC_CACHE = _build()
    return _NC_CACHE


def _pack_x(xb):
    """[T, E] batch slice -> slice-major packed [128, ECH*T] bf16 (xT layout)."""
    xT = xb.T.reshape(ECH, 128, T // 512, 512)          # [e, p, s, t']
    return np.ascontiguousarray(
        xT.transpose(1, 2, 0, 3).reshape(128, ECH * T)).astype(ml_dtypes.bfloat16)


def _pack_w(w_sl):
    """[384, 768] weight slice -> e-major packed [128, ECH*OC] bf16 (for wv:
    rhs slice for e-chunk at cols [OC*e, OC*(e+1)))."""
    wT = w_sl.T.reshape(ECH, 128, OC)                   # [e, p, j]
    return np.ascontiguousarray(
        wT.transpose(1, 0, 2).reshape(128, ECH * OC)).astype(ml_dtypes.bfloat16)


def _pack_w_cm(w_sl):
    """[384, 768] weight slice -> chunk-major packed [128, ECH*OC] bf16:
    lhsT for (chunk c, e-chunk e) at cols [768c+128e, 768c+128e+128)."""
    wT = w_sl.T.reshape(ECH, 128, QKC, 128)             # [e, p, c, j]
    return np.ascontiguousarray(
        wT.transpose(1, 2, 0, 3).reshape(128, ECH * OC)).astype(ml_dtypes.bfloat16)


def kernel(hidden_states, attention_mask, Wq, bq, Wk, bk, Wv, bv):
    nc = _get_nc()
    in_maps = _make_in_maps(hidden_states, attention_mask, Wq, bq, Wk, bk, Wv, bv)
    res = bass_utils.run_bass_kernel_spmd(nc, in_maps, core_ids=list(range(NCORES)))
    return _assemble(res.results)


def _make_in_maps(hidden_states, attention_mask, Wq, bq, Wk, bk, Wv, bv):
    hidden_states = np.asarray(hidden_states, dtype=np.float32)
    attention_mask = np.asarray(attention_mask, dtype=np.float32)
    Wq, Wk, Wv = (np.asarray(w, dtype=np.float32) for w in (Wq, Wk, Wv))
    bq, bk, bv = (np.asarray(b, dtype=np.float32) for b in (bq, bk, bv))

    # causal bias for the diagonal block, duplicated for the two head halves:
    # trin[s, j*128 + t] = MNEG where key s > query t
    tri1 = np.where(np.arange(128)[:, None] > np.arange(128)[None, :],
                    np.float32(MNEG), np.float32(0.0))
    trin = np.concatenate([tri1, tri1], axis=1)         # [128, 256]

    in_maps = []
    for c in range(NCORES):
        b, g = divmod(c, 2)
        sl = slice(OC * g, OC * (g + 1))
        m = attention_mask[b, 0, 0, :]
        w = np.exp(m).astype(np.float32)                # per-key weight
        # wpk[p, 8*tb + j] = w[128*tb + p] for j in 0..6 (col 7 pad)
        wpk = np.zeros((128, 8 * NSB), np.float32)
        wblk = w.reshape(NSB, 128).T                    # [p, tb]
        for j in range(7):
            wpk[:, j::8] = wblk
        in_maps.append({
            "xTp": _pack_x(hidden_states[b]),
            "wqp": _pack_w_cm(Wq[sl]),
            "wkp": _pack_w_cm(Wk[sl]),
            "wvp": _pack_w(Wv[sl]),
            "bq": np.ascontiguousarray(bq[sl]).reshape(QKC, 128, 1),
            "bk": np.ascontiguousarray(bk[sl]).reshape(QKC, 128, 1),
            "bvr": np.broadcast_to(bv[sl], (128, OC)).copy(),
            "wpk": wpk,
            "trin": trin,
        })
    return in_maps


def _assemble(results):
    out = np.empty((B, T, E), np.float32)
    for c in range(NCORES):
        b, g = divmod(c, 2)
        oT = results[c]["outT"]  # [6, 65, 2048]
        for h6 in range(HPC):
            h = HPC * g + h6
            out[b, :, D * h:D * h + D] = (oT[h6, :D] / oT[h6, D:D + 1]).T
    return out


# revision 25
# speedup vs baseline: 1.0479x; 1.0479x over previous
"""Causal self-attention (B=4, T=2048, E=768, H=12, D=64) on 8 TRN2 NeuronCores.

Sharding: core c handles batch b = c//2 and head-group g = c%2 (6 heads each).
Per core:
    qT, kT = (x @ WqT + bq).T, ...        stored [384, 2048] (heads x 64, T)
    v      = x @ WvT + bv, scaled by w=exp(mask), stored fp8 in strip-pair
             interleaved tiles [128, 2 strips, 6 heads x 65] with a w column
    per head-pair, per key-strip sb (128 keys), per 512-col piece:
        scoresT[s, t] for BOTH heads into ONE psum tile [128, 1024]
        (h0 cols 0-511, h1 cols 512-1023) via four 64x64-tiled matmuls that
        run concurrently in the PE array.
        causal mask: DVE adds -1920 to the masked lanes of the diagonal
        128x128 block directly in psum (exp then underflows to 0).
        ONE exp op per piece covers both heads through a strided 3D out AP,
        writing fp8e4 into the strip-pair interleaved expT buffer.
    per head, per t-window (512 cols): PV via fp8 DoubleRow matmuls, one per
        strip-PAIR (256-wide contraction: v tiles hold strip pairs at j-stride
        400, expT holds them at j-stride Wp). Psum [65, W] accumulates over
        pairs; row 64 (w column) is the softmax denominator. Window result is
        DMA'd to DRAM directly from psum.
Host: output[b, :, h*64:(h+1)*64] = (outT_h[:64] / outT_h[64:65]).T

attention_mask is folded in as w_s = exp(mask_s): v' = w*v and the ones
column becomes w, so softmax numerator and denominator both carry w exactly.
All matmul start=True writes begin on a PSUM bank boundary. Inputs are
host-packed into the exact SBUF layouts so every input lands in a handful of
contiguous 2D DMAs on two queues.
"""

import numpy as np
import ml_dtypes

import concourse.bacc as bacc
import concourse.mybir as mybir
import concourse.tile as tile
from concourse import bass_utils

F32 = mybir.dt.float32
BF16 = mybir.dt.bfloat16
FP8 = mybir.dt.float8e4

B, T, E, H, D = 4, 2048, 768, 12, 64
NCORES = 8
HPC = 6             # heads per core
OC = HPC * D        # 384 output channels per core
ECH = E // 128      # 6 contraction chunks
QKC = OC // 128     # 3 qT/kT partition chunks (= head pairs)
NSB = T // 128      # 16 key strips
NPB = NSB // 2      # 8 strip pairs
SCALE = 0.125       # 1/sqrt(D)
XSL = ECH * 512     # xT free elems per t-slice (3072)
VJS = 400           # vtp j-stride (>= 6*65, 16B-aligned for DoubleRow)
MNEG = -1920.0      # causal mask bias: SCALE*(-1920) = -240 -> exp == 0

# strip-pair expT regions: pair pb holds strips (2pb, 2pb+1) for both heads
# of the current attention pair. Region for (pb, hh) is [2, Wp] fp8 at
# OFF[pb] + 2*Wp*hh; j-row 1 (odd strip) is absolute-t aligned, so its first
# 128 cols are a never-written zero pad.
PW = [T - 256 * pb for pb in range(NPB)]
OFF = [0] * (NPB + 1)
for _pb in range(NPB):
    OFF[_pb + 1] = OFF[_pb] + 4 * PW[_pb]
EXBW = OFF[NPB]  # 36864


def _strip_pieces(sb):
    """(piece_idx, start, width) pieces of strip sb on the 512 grid."""
    W = T - 128 * sb
    return [(p // 512, p, min(512, W - p)) for p in range(0, W, 512)]


def _xoff(e, t):
    """Free-dim offset of (e-chunk, t) in the slice-major packed xT tile."""
    return (t // 512) * XSL + 512 * e + (t % 512)


def _build():
    nc = bacc.Bacc("TRN2", debug=False)

    xT_d = nc.dram_tensor("xTp", [128, ECH * T], BF16, kind="ExternalInput")
    wq_d = nc.dram_tensor("wqp", [128, ECH * OC], BF16, kind="ExternalInput")
    wk_d = nc.dram_tensor("wkp", [128, ECH * OC], BF16, kind="ExternalInput")
    wv_d = nc.dram_tensor("wvp", [128, ECH * OC], BF16, kind="ExternalInput")
    bq_d = nc.dram_tensor("bq", [QKC, 128, 1], F32, kind="ExternalInput")
    bk_d = nc.dram_tensor("bk", [QKC, 128, 1], F32, kind="ExternalInput")
    bvr_d = nc.dram_tensor("bvr", [128, OC], F32, kind="ExternalInput")
    wpk_d = nc.dram_tensor("wpk", [128, 8 * NSB], F32, kind="ExternalInput")
    trin_d = nc.dram_tensor("trin", [128, 256], F32, kind="ExternalInput")
    out_d = nc.dram_tensor("outT", [HPC, D + 1, T], F32, kind="ExternalOutput")

    with tile.TileContext(nc) as tc:
        with (
            tc.tile_pool(name="persist", bufs=1) as pp,
            tc.tile_pool(name="qk_ps", bufs=3, space="PSUM") as qk_ps,
            tc.tile_pool(name="b1_ps", bufs=2, space="PSUM") as b1_ps,
            tc.tile_pool(name="stage", bufs=4) as stage,
        ):
            # ---- persistent SBUF tensors ----
            xt_all = pp.tile([128, ECH * T], BF16, tag="xt", name="xt")
            wq_all = pp.tile([128, ECH * OC], BF16, tag="wq", name="wq")
            wk_all = pp.tile([128, ECH * OC], BF16, tag="wk", name="wk")
            wv_all = pp.tile([128, ECH * OC], BF16, tag="wv", name="wv")
            wv = [wv_all[:, OC * e:OC * (e + 1)] for e in range(ECH)]

            def wqk_sl(w_all, c, e):
                o = 768 * c + 128 * e
                return w_all[:, o:o + 128]
            qt = [pp.tile([128, T], BF16, tag=f"qt{c}", name=f"qt{c}") for c in range(QKC)]
            kt = [pp.tile([128, T], BF16, tag=f"kt{c}", name=f"kt{c}") for c in range(QKC)]
            vtp = [pp.tile([128, 2, VJS], FP8, tag=f"vt{p}", name=f"vt{p}") for p in range(NPB)]
            # fp8 residuals of v for pairs 0-1: corrects the first t-window,
            # where attention is peaked and out ~= v so v's fp8 rounding
            # would land directly in the output
            vlo = [pp.tile([128, 2, VJS], FP8, tag=f"vl{p}", name=f"vl{p}") for p in range(2)]
            # bf16 side path for output cols t < 256: with few keys the fp8
            # rounding of exp perturbs softmax weights directly (no averaging)
            vt16 = [pp.tile([128, HPC, 65], BF16, tag=f"v6{p}", name=f"v6{p}") for p in range(2)]
            exb16 = pp.tile([128, 2, 384], BF16, tag="ex6", name="ex6")
            exb = pp.tile([128, EXBW], FP8, tag="exb", name="exb")
            bq_t = [pp.tile([128, 1], F32, tag=f"bq{c}", name=f"bq{c}") for c in range(QKC)]
            bk_t = [pp.tile([128, 1], F32, tag=f"bk{c}", name=f"bk{c}") for c in range(QKC)]
            bvr_t = pp.tile([128, OC], F32, tag="bvr", name="bvr")
            wpk_t = pp.tile([128, 8 * NSB], F32, tag="wpk", name="wpk")
            trin_t = pp.tile([128, 256], F32, tag="trin", name="trin")

            # ---- input DMAs: contiguous 2D transfers on two hardware queues
            # (sync + scalar); xT slice-major so projections start early ----
            def dma_w(eng, w_all, w_d, c):
                eng.dma_start(w_all[:, 768 * c:768 * c + 768],
                              w_d.ap()[:, 768 * c:768 * c + 768])

            def dma_x(eng, t0):
                s0 = (t0 // 512) * XSL
                eng.dma_start(xt_all[:, s0:s0 + XSL], xT_d.ap()[:, s0:s0 + XSL])

            def dma_xh(eng, t0, half):
                # half a 512-col t-slice of xT (splitting the first slices
                # across two queues halves the time to the first projection)
                s0 = (t0 // 512) * XSL + half * (XSL // 2)
                eng.dma_start(xt_all[:, s0:s0 + XSL // 2],
                              xT_d.ap()[:, s0:s0 + XSL // 2])

            # one-time zero pad first (no input deps): j=1 row's first 128
            # cols of every region
            for pb in range(NPB):
                for hh in (0, 1):
                    o = OFF[pb] + 2 * PW[pb] * hh + PW[pb]
                    nc.vector.memset(exb[:, o:o + 128], 0.0)
            # zero v tiles: stale fp8 bytes can encode NaN, and NaN*0 = NaN
            # even where the exp operand is a structural zero
            for t_ in vtp + vlo:
                nc.vector.memset(t_[:, :, :], 0.0)

            # DMA issues block once a queue has >4 in flight, so the scalar
            # (ACT) queue gets only two issues; sync and gpsimd (idle early)
            # carry the rest, ordered by when the data is first needed.
            dma_w(nc.sync, wk_all, wk_d, 0)
            dma_xh(nc.sync, 0, 0)
            dma_w(nc.scalar, wq_all, wq_d, 0)
            dma_xh(nc.scalar, 0, 1)
            nc.gpsimd.dma_start(trin_t[:, :], trin_d.ap()[:, :])
            for c in range(QKC):
                nc.gpsimd.dma_start(bq_t[c][:, :], bq_d.ap()[c])
                nc.gpsimd.dma_start(bk_t[c][:, :], bk_d.ap()[c])
            nc.gpsimd.dma_start(wpk_t[:, :], wpk_d.ap()[:, :])
            dma_xh(nc.sync, 512, 0)
            dma_xh(nc.gpsimd, 512, 1)
            dma_w(nc.sync, wk_all, wk_d, 1)
            dma_w(nc.gpsimd, wq_all, wq_d, 1)
            dma_xh(nc.sync, 1024, 0)
            dma_xh(nc.gpsimd, 1024, 1)
            nc.gpsimd.dma_start(wv_all[:, :], wv_d.ap()[:, :])
            nc.gpsimd.dma_start(bvr_t[:, :], bvr_d.ap()[:, :])
            dma_xh(nc.sync, 1536, 0)
            dma_xh(nc.gpsimd, 1536, 1)
            dma_w(nc.sync, wk_all, wk_d, 2)
            dma_w(nc.gpsimd, wq_all, wq_d, 2)

            def proj_qk_chain(c, t0, which):
                # one 512-col chain of the qT or kT projection for chunk c
                w_all, dst, bias = ((wk_all, kt, bk_t), (wq_all, qt, bq_t))[which]
                ps = b1_ps.tile([128, 512], F32, tag="b1", name="pp")
                for e in range(ECH):
                    nc.tensor.matmul(
                        ps[:, :],
                        wqk_sl(w_all, c, e),
                        xt_all[:, _xoff(e, t0):_xoff(e, t0) + 512],
                        start=(e == 0), stop=(e == ECH - 1),
                    )
                nc.vector.tensor_scalar_add(
                    dst[c][:, t0:t0 + 512], ps[:, :], bias[c][:, 0:1])

            def proj_v_chain(tb):
                pb, dlt = divmod(tb, 2)
                ps = b1_ps.tile([128, OC], F32, tag="b1", name="ppv")
                for e in range(ECH):
                    o = _xoff(e, 128 * tb)
                    nc.tensor.matmul(
                        ps[:, :],
                        xt_all[:, o:o + 128],
                        wv[e][:, :],
                        start=(e == 0), stop=(e == ECH - 1),
                    )
                # v' = (psum + bv) * w, fp8; w column = w  (w = exp(mask))
                nc.vector.tensor_tensor(
                    ps[:, :], ps[:, :], bvr_t[:, :], op=mybir.AluOpType.add)
                nc.vector.tensor_scalar_mul(
                    ps[:, :], ps[:, :], wpk_t[:, 8 * tb:8 * tb + 1])
                dst = vtp[pb][:, dlt, 0:HPC * 65].rearrange(
                    "p (h d) -> p h d", h=HPC)
                nc.vector.tensor_copy(
                    dst[:, :, 0:D], ps.rearrange("p (h d) -> p h d", h=HPC))
                nc.vector.tensor_copy(
                    dst[:, :, D:D + 1],
                    wpk_t[:, 8 * tb + 1:8 * tb + 7].rearrange(
                        "p (h d) -> p h d", d=1))
                if tb < 4:
                    # residual: vlo = fp8(v' - float(fp8(v')))
                    lo = vlo[pb][:, dlt, 0:HPC * 65].rearrange(
                        "p (h d) -> p h d", h=HPC)
                    nc.vector.tensor_tensor(
                        lo[:, :, 0:D],
                        ps.rearrange("p (h d) -> p h d", h=HPC),
                        dst[:, :, 0:D], op=mybir.AluOpType.subtract)
                if tb < 2:
                    nc.vector.tensor_copy(
                        vt16[tb][:, :, 0:D],
                        ps.rearrange("p (h d) -> p h d", h=HPC))
                    nc.vector.tensor_copy(
                        vt16[tb][:, :, D:D + 1],
                        wpk_t[:, 8 * tb + 1:8 * tb + 7].rearrange(
                            "p (h d) -> p h d", d=1))

            def qk_exp_piece(h0, sb, p, rp, pw):
                # piece p of strip sb for the pair (h0, h0+1): two adjacent
                # head blocks in one psum tile (h0 cols 0-511, h1 512-1023)
                # via 4 concurrent 64x64-tiled matmuls, then causal-mask the
                # diagonal block in psum (p==0) and ONE strided exp -> fp8.
                c = h0 // 2
                t0 = 128 * sb
                ps = qk_ps.tile([128, 1024], F32, tag="qk", name="qk")
                for ofs, rows in ((0, slice(0, 64)), (512, slice(64, 128))):
                    for so, pr in ((0, slice(0, 64)), (64, slice(64, 128))):
                        nc.tensor.matmul(
                            ps[pr, ofs:ofs + pw],
                            kt[c][rows, t0 + so:t0 + so + 64],
                            qt[c][rows, t0 + rp:t0 + rp + pw],
                            start=True, stop=True,
                        )
                ps3 = ps.rearrange("p (h c) -> p h c", h=2)
                if p == 0:
                    # diag 128x128 blocks (cols 0-127 and 512-639): add -1920
                    # where key > query so exp underflows to exact 0
                    nc.vector.tensor_tensor(
                        ps3[:, :, 0:128], ps3[:, :, 0:128],
                        trin_t.rearrange("p (h c) -> p h c", h=2),
                        op=mybir.AluOpType.add)
                pb, dlt = divmod(sb, 2)
                Wp = PW[pb]
                d0 = dlt * (Wp + 128) + rp
                dst = exb[:, OFF[pb]:OFF[pb] + 4 * Wp].rearrange(
                    "p (h c) -> p h c", h=2)[:, :, d0:d0 + pw]
                nc.scalar.activation(
                    dst, ps3[:, :, 0:pw],
                    mybir.ActivationFunctionType.Exp, scale=SCALE)
                if sb <= 1 and p == 0:
                    # bf16 exp copy of the t < 256 cols for the side path
                    w16 = 256 if sb == 0 else 128
                    o16 = 0 if sb == 0 else 256
                    nc.scalar.activation(
                        exb16[:, :, o16:o16 + w16], ps3[:, :, 0:w16],
                        mybir.ActivationFunctionType.Exp, scale=SCALE)

            _oq = [0]

            def out_queue():
                _oq[0] += 1
                return (nc.sync, nc.scalar, nc.gpsimd)[_oq[0] % 3]

            def pv_window(h, c0, W):
                # one [65, W] psum accumulation over strip-pairs via fp8
                # DoubleRow (256-wide contraction), then DMA straight out.
                hh = h % 2
                ps = b1_ps.tile([D + 1, W], F32, tag="b1", name="pv",
                                padded_shape=[D + 1, 512])
                segs = [(pb, vtp[pb]) for pb in range(NPB) if 256 * pb < c0 + W]
                if c0 < 512:
                    # residual correction for the first windows (peaked rows)
                    segs += [(pb, vlo[pb]) for pb in range(2) if 256 * pb < c0 + W]
                for i, (pb, vt_) in enumerate(segs):
                    s0 = max(c0, 256 * pb)
                    w = c0 + W - s0
                    Wp = PW[pb]
                    o = OFF[pb] + 2 * Wp * hh
                    rhs = exb[:, o:o + 2 * Wp].rearrange(
                        "p (j c) -> p j c", j=2)[:, :, s0 - 256 * pb:s0 - 256 * pb + w]
                    nc.tensor.matmul(
                        ps[:, s0 - c0:s0 - c0 + w],
                        vt_[:, :, 65 * h:65 * h + 65],
                        rhs,
                        start=(i == 0), stop=(i == len(segs) - 1),
                        perf_mode=mybir.MatmulPerfMode.DoubleRow,
                    )
                st = stage.tile([D + 1, W], F32, tag="st", name="st",
                                padded_shape=[D + 1, 512])
                nc.vector.tensor_copy(st[:, :], ps[:, :])
                out_queue().dma_start(out_d.ap()[h, :, c0:c0 + W], st[:, :])

            def pv16_window(h):
                # bf16 side path for output cols [0, 256): strips 0 and 1
                hh = h % 2
                ps = b1_ps.tile([D + 1, 256], F32, tag="b1", name="pv6",
                                padded_shape=[D + 1, 512])
                nc.tensor.matmul(
                    ps[:, 0:256], vt16[0][:, h, :], exb16[:, hh, 0:256],
                    start=True, stop=False)
                nc.tensor.matmul(
                    ps[:, 128:256], vt16[1][:, h, :], exb16[:, hh, 256:384],
                    start=False, stop=True)
                st = stage.tile([D + 1, 256], F32, tag="st", name="st6",
                                padded_shape=[D + 1, 512])
                nc.vector.tensor_copy(st[:, :], ps[:, :])
                out_queue().dma_start(out_d.ap()[h, :, 0:256], st[:, :])

            # pv windows: strip -> [(c0, W)]. Tail windows split so most of
            # the last chunk's accumulation runs before the final strips.
            PV_WINDOWS = {3: [(256, 256)], 7: [(512, 512)], 11: [(1024, 512)],
                          13: [(1536, 256)], 14: [(1792, 128)],
                          15: [(1920, 128)]}

            # ---- pipelined emission: a global slot pipeline ----
            # Pair p's strips occupy slots [12p, 12p+16); consecutive pairs
            # overlap by 4 slots so ACT never idles at pair boundaries. A
            # piece whose expT region is still to be read by the previous
            # pair's late pv windows (t >= 1536) is deferred past them.
            NSLOT = 41
            slot_work = [[] for _ in range(NSLOT)]   # items: (kind, fn)
            # kind 0: scores pieces; 1: proj fillers; 2: pv windows

            # pair (0,1) cascaded start: emit chunk-0 projection chains
            # t-ascending; after each chain emit every scores piece whose q/k
            # columns are available, so the first exp fires early.
            all_pieces = [(128 * sb + rp + pw, sb, p, rp, pw)
                          for sb in range(NSB)
                          for p, rp, pw in _strip_pieces(sb)]
            all_pieces.sort(key=lambda x: (x[0], x[1]))
            emitted = set()

            def emit_ready(limit):
                for need, sb, p, rp, pw in all_pieces:
                    if need > limit:
                        break
                    if (sb, p) in emitted:
                        continue
                    qk_exp_piece(0, sb, p, rp, pw)
                    emitted.add((sb, p))

            # chunk-0 chains for t0 <= 1024 cascade inline; the t0=1536 pair
            # is deferred to slot 3 (its x slice is last off the wire), with
            # the pieces that need it deferred to slot >= 4, so the PE queue
            # never stalls on the late DMA while ready work sits behind it.
            for t0 in (0, 512, 1024):
                proj_qk_chain(0, t0, 0)
                proj_qk_chain(0, t0, 1)
                emit_ready(min(t0 + 512, 1024))

            def sched_pair(h0, base, defer_slot):
                for sb in range(NSB):
                    for p, rp, pw in _strip_pieces(sb):
                        if h0 == 0 and (sb, p) in emitted:
                            continue
                        t_end = 128 * sb + rp + pw
                        sl = base + sb
                        if t_end > 1536 and sl < defer_slot:
                            sl = defer_slot
                        slot_work[sl].append(
                            (0, lambda h0=h0, sb=sb, p=p, rp=rp, pw=pw:
                             qk_exp_piece(h0, sb, p, rp, pw)))
                    if sb == 2:
                        slot_work[base + 2].append(
                            (2, lambda h0=h0: pv16_window(h0)))
                        slot_work[base + 2].append(
                            (2, lambda h0=h0: pv16_window(h0 + 1)))
                    for tc0, W in PV_WINDOWS.get(sb, ()):
                        sl = max(base + sb, defer_slot if base else 0)
                        slot_work[sl].append(
                            (2, lambda h0=h0, tc0=tc0, W=W: pv_window(h0, tc0, W)))
                        slot_work[sl].append(
                            (2, lambda h0=h0, tc0=tc0, W=W: pv_window(h0 + 1, tc0, W)))

            slot_work[3].append((0, lambda: proj_qk_chain(0, 1536, 0)))
            slot_work[3].append((0, lambda: proj_qk_chain(0, 1536, 1)))
            sched_pair(0, 0, 4)
            sched_pair(2, 12, 16)
            sched_pair(4, 24, 28)
            # fillers: v projection paced over pair01's strips (1/slot, just
            # ahead of the pv windows); chunk-1/2 q/k projections early
            # block 15 goes to slot 14: pair 0's [1792,1920) window (slot 14)
            # touches vtp[7] (its exp side is the structural zero pad, but the
            # v bytes must be initialized before any matmul reads them)
            for tb in range(NSB):
                slot_work[min(tb, 14)].append((1, lambda tb=tb: proj_v_chain(tb)))
            for i in range(8):
                t0, wch = (i % 4) * 512, i // 4
                slot_work[1 + i].append(
                    (1, lambda t0=t0, w=wch: proj_qk_chain(1, t0, w)))
                slot_work[13 + i].append(
                    (1, lambda t0=t0, w=wch: proj_qk_chain(2, t0, w)))

            for sl in range(NSLOT):
                for _, f in sorted(slot_work[sl], key=lambda kf: kf[0]):
                    f()

    nc.compile()
    return nc


_NC_CACHE = None


def _get_nc():
    global _NC_CACHE
    if _NC_CACHE is None:
        _NC_CACHE = _build()
    return _NC_CACHE


def _pack_x(xb):
    """[T, E] batch slice -> slice-major packed [128, ECH*T] bf16 (xT layout)."""
    xT = xb.T.reshape(ECH, 128, T // 512, 512)          # [e, p, s, t']
    return np.ascontiguousarray(
        xT.transpose(1, 2, 0, 3).reshape(128, ECH * T)).astype(ml_dtypes.bfloat16)


def _pack_w(w_sl):
    """[384, 768] weight slice -> e-major packed [128, ECH*OC] bf16 (for wv:
    rhs slice for e-chunk at cols [OC*e, OC*(e+1)))."""
    wT = w_sl.T.reshape(ECH, 128, OC)                   # [e, p, j]
    return np.ascontiguousarray(
        wT.transpose(1, 0, 2).reshape(128, ECH * OC)).astype(ml_dtypes.bfloat16)


def _pack_w_cm(w_sl):
    """[384, 768] weight slice -> chunk-major packed [128, ECH*OC] bf16:
    lhsT for (chunk c, e-chunk e) at cols [768c+128e, 768c+128e+128)."""
    wT = w_sl.T.reshape(ECH, 128, QKC, 128)             # [e, p, c, j]
    return np.ascontiguousarray(
        wT.transpose(1, 2, 0, 3).reshape(128, ECH * OC)).astype(ml_dtypes.bfloat16)


def kernel(hidden_states, attention_mask, Wq, bq, Wk, bk, Wv, bv):
    nc = _get_nc()
    in_maps = _make_in_maps(hidden_states, attention_mask, Wq, bq, Wk, bk, Wv, bv)
    res = bass_utils.run_bass_kernel_spmd(nc, in_maps, core_ids=list(range(NCORES)))
    return _assemble(res.results)


def _make_in_maps(hidden_states, attention_mask, Wq, bq, Wk, bk, Wv, bv):
    hidden_states = np.asarray(hidden_states, dtype=np.float32)
    attention_mask = np.asarray(attention_mask, dtype=np.float32)
    Wq, Wk, Wv = (np.asarray(w, dtype=np.float32) for w in (Wq, Wk, Wv))
    bq, bk, bv = (np.asarray(b, dtype=np.float32) for b in (bq, bk, bv))

    # causal bias for the diagonal block, duplicated for the two head halves:
    # trin[s, j*128 + t] = MNEG where key s > query t
    tri1 = np.where(np.arange(128)[:, None] > np.arange(128)[None, :],
                    np.float32(MNEG), np.float32(0.0))
    trin = np.concatenate([tri1, tri1], axis=1)         # [128, 256]

    in_maps = []
    for c in range(NCORES):
        b, g = divmod(c, 2)
        sl = slice(OC * g, OC * (g + 1))
        m = attention_mask[b, 0, 0, :]
        w = np.exp(m).astype(np.float32)                # per-key weight
        # wpk[p, 8*tb + j] = w[128*tb + p] for j in 0..6 (col 7 pad)
        wpk = np.zeros((128, 8 * NSB), np.float32)
        wblk = w.reshape(NSB, 128).T                    # [p, tb]
        for j in range(7):
            wpk[:, j::8] = wblk
        in_maps.append({
            "xTp": _pack_x(hidden_states[b]),
            "wqp": _pack_w_cm(Wq[sl]),
            "wkp": _pack_w_cm(Wk[sl]),
            "wvp": _pack_w(Wv[sl]),
            "bq": np.ascontiguousarray(bq[sl]).reshape(QKC, 128, 1),
            "bk": np.ascontiguousarray(bk[sl]).reshape(QKC, 128, 1),
            "bvr": np.broadcast_to(bv[sl], (128, OC)).copy(),
            "wpk": wpk,
            "trin": trin,
        })
    return in_maps


def _assemble(results):
    out = np.empty((B, T, E), np.float32)
    for c in range(NCORES):
        b, g = divmod(c, 2)
        oT = results[c]["outT"]  # [6, 65, 2048]
        for h6 in range(HPC):
            h = HPC * g + h6
            out[b, :, D * h:D * h + D] = (oT[h6, :D] / oT[h6, D:D + 1]).T
    return out
